# revision 1
# baseline (speedup 1.0000x reference)
"""RWKV-7 TimeMix kernel for 8 Trainium2 NeuronCores.

Sharding: data-parallel over B (8 batches -> 8 cores). Each core runs the
full per-batch module: time-shift lerps, r/k/v projections, LoRA branches
(decay/iclr/gate), the WKV state recurrence (chunked UT-transform with a
truncated-Neumann intra-chunk solve), bonus, GroupNorm, output gate, W_o.

Chunked WKV math per head (chunk L=128, state S[i,j], c = in-chunk cumprod
of the decay d):
  Wt[t] = a_t*kn_t*c_{t-1}      Kn[s] = kn_s/c_s       Vs[s] = v_s/c_s
  G  = triu(Kn Wt^T, 1)   Av = triu(Vs Wt^T, 1)   P = triu(kr r^T, 0)
  B  = Wt S0 + Av^T kr
  (I+G^T) U = B  solved by K Neumann steps  X <- B - G^T X;  Um := -U
  Qm = triu(Um r^T, 0)
  O[t] = c_t * (r S0^T + P^T Vs + Qm^T Kn)
  S   <- diag(c_L) (S + Vs^T kr + Kn^T Um)
Value-path matmuls run fp32 (exact); the G path (G generation + Neumann
applies) runs bf16; big projections run float32r (tf32-like, full speed).
W_k/W_v/W_o are streamed from HBM per super-chunk; W_r stays resident.
"""
import numpy as np

B, T, C, H, N = 8, 2048, 1024, 16, 64
LORA = 64
P = 128
NCT = C // P          # 8 channel tiles
CH = 128              # WKV chunk
SUP = 256             # projection super-chunk
NSUP = T // SUP       # 8
NCH = SUP // CH       # 2
NEUMANN_K = 6
DECAY_SCALE = float(np.exp(-0.5))
GN_EPS = 1e-5 * H
NORM_EPS = 1e-12

VEC_NAMES = ["mu_r", "mu_k", "mu_v", "mu_g", "mu_a", "mu_d",
             "decay_bias", "iclr_bias", "removal_key_multiplier",
             "iclr_mix_amt", "bonus_multiplier", "ln_w", "ln_b"]
MAT_NAMES = ["W_r", "W_k", "W_v", "W_o", "decay_A", "iclr_A", "gate_A",
             "decay_B", "iclr_B", "gate_B"]

_CACHE = {}


def _build():
    import concourse.bass as bass  # noqa: F401
    from concourse import bacc, mybir
    import concourse.tile as tile

    f32 = mybir.dt.float32
    nc = bacc.Bacc("TRN2", target_bir_lowering=False, debug=False, num_devices=B)
    x_h = nc.dram_tensor("x", [T, C], f32, kind="ExternalInput")
    w_h = {n: nc.dram_tensor(n, [C, C], f32, kind="ExternalInput")
           for n in ("W_r", "W_k", "W_v", "W_o")}
    la_h = {n: nc.dram_tensor(n, [C, LORA], f32, kind="ExternalInput")
            for n in ("decay_A", "iclr_A", "gate_A")}
    lb_h = {n: nc.dram_tensor(n, [LORA, C], f32, kind="ExternalInput")
            for n in ("decay_B", "iclr_B", "gate_B")}
    vec_h = {n: nc.dram_tensor(n, [C], f32, kind="ExternalInput") for n in VEC_NAMES}
    y_h = nc.dram_tensor("y", [T, C], f32, kind="ExternalOutput")
    vp_h = nc.dram_tensor("vp", [T, C], f32, kind="ExternalOutput")
    with tile.TileContext(nc) as tc:
        _emit(nc, tc, x_h, w_h, la_h, lb_h, vec_h, y_h, vp_h)
    nc.finalize()
    return nc


def _emit(nc, tc, x_h, w_h, la_h, lb_h, vec_h, y_h, vp_h):
    import concourse.bass as bass
    from concourse import mybir
    from concourse.masks import make_identity
    from contextlib import ExitStack

    f32 = mybir.dt.float32
    f32r = mybir.dt.float32r
    bf16 = mybir.dt.bfloat16
    AF = mybir.ActivationFunctionType
    OP = mybir.AluOpType

    ctx = ExitStack()
    const = ctx.enter_context(tc.tile_pool(name="const", bufs=1))
    supp = ctx.enter_context(tc.tile_pool(name="supp", bufs=1))
    chkp = ctx.enter_context(tc.tile_pool(name="chkp", bufs=1))
    jit1 = ctx.enter_context(tc.tile_pool(name="jit1", bufs=1))
    jit2 = ctx.enter_context(tc.tile_pool(name="jit2", bufs=2))
    jit3 = ctx.enter_context(tc.tile_pool(name="jit3", bufs=3))
    pairp = ctx.enter_context(tc.tile_pool(name="pairp", bufs=2))
    ps_proj = ctx.enter_context(tc.tile_pool(name="ps_proj", bufs=2, space="PSUM"))
    ps_lora = ctx.enter_context(tc.tile_pool(name="ps_lora", bufs=1, space="PSUM"))
    ps_wkv = ctx.enter_context(tc.tile_pool(name="ps_wkv", bufs=3, space="PSUM"))
    ps_small = ctx.enter_context(tc.tile_pool(name="ps_small", bufs=2, space="PSUM"))

    # ---------------- static constants ----------------
    wr_sb = const.tile([P, NCT, C], f32r, tag="wr_sb")
    for i in range(NCT):
        nc.gpsimd.dma_start(out=wr_sb[:, i, :], in_=w_h["W_r"][P * i:P * (i + 1), :])
    vsb = {}
    for n in VEC_NAMES:
        t = const.tile([P, NCT], f32, tag=f"v_{n}", name=f"v_{n}")
        src = vec_h[n][:]
        nc.sync.dma_start(out=t, in_=bass.AP(
            tensor=src.tensor, offset=src.offset, ap=[[1, P], [P, NCT]]))
        vsb[n] = t
    ommix = const.tile([P, NCT], f32, tag="v_ommix")
    nc.vector.tensor_scalar(out=ommix, in0=vsb["iclr_mix_amt"], scalar1=-1.0,
                            scalar2=1.0, op0=OP.mult, op1=OP.add)
    ident = const.tile([P, P], f32, tag="ident")
    make_identity(nc, ident)
    mask_su = const.tile([P, P], f32, tag="mask_su")   # keep s < t  ([s,t])
    nc.gpsimd.memset(mask_su, 1.0)
    nc.gpsimd.affine_select(out=mask_su, in_=mask_su, compare_op=OP.is_gt,
                            fill=0.0, base=0, channel_multiplier=-1,
                            pattern=[[1, P]])
    mask_ui = const.tile([P, P], f32, tag="mask_ui")   # keep s <= t
    nc.gpsimd.memset(mask_ui, 1.0)
    nc.gpsimd.affine_select(out=mask_ui, in_=mask_ui, compare_op=OP.is_ge,
                            fill=0.0, base=0, channel_multiplier=-1,
                            pattern=[[1, P]])
    inds = []
    for i in range(NCT):
        indf = const.tile([P, 16], f32, tag=f"indf{i}", name=f"indf{i}")
        nc.vector.memset(indf, 0.0)
        nc.vector.memset(indf[0:64, 2 * i:2 * i + 1], 1.0)
        nc.vector.memset(indf[64:128, 2 * i + 1:2 * i + 2], 1.0)
        indr = const.tile([P, 16], f32r, tag=f"indr{i}", name=f"indr{i}")
        nc.scalar.copy(out=indr, in_=indf)
        inds.append(indr)
    gn_eps = const.tile([P, 1], f32, tag="gn_eps")
    nc.vector.memset(gn_eps, GN_EPS)
    zeros = const.tile([P, CH], f32, tag="zeros")
    nc.vector.memset(zeros, 0.0)
    S_st = const.tile([64, NCT, 2, 64], f32, tag="S_st")
    St_st = const.tile([64, NCT, 2, 64], f32, tag="St_st")
    nc.vector.memset(S_st, 0.0)
    nc.vector.memset(St_st, 0.0)

    # ------------- per-super persistents -------------
    xext = supp.tile([P, NCT, SUP + 1], f32, tag="xext")
    rT = supp.tile([P, NCT, SUP], f32, tag="rT")
    knT = supp.tile([P, NCT, SUP], f32, tag="knT")
    krT = supp.tile([P, NCT, SUP], f32, tag="krT")
    vT = supp.tile([P, NCT, SUP], f32, tag="vT")
    aT = supp.tile([P, NCT, SUP], f32, tag="aT")
    dT = supp.tile([P, NCT, SUP], f32, tag="dT")
    gateT = supp.tile([P, NCT, SUP], f32, tag="gateT")
    yTs = supp.tile([P, NCT, SUP], f32r, tag="yTs")
    la_out = {n: supp.tile([LORA, SUP], f32r, tag=f"lo_{n}", name=f"lo_{n}")
              for n in ("decay_A", "iclr_A", "gate_A")}
    nb = supp.tile([16, 2 * SUP], f32, tag="nb")
    # ------------- per-chunk persistents -------------
    cext = chkp.tile([P, NCT, CH + 1], f32, tag="cext")
    knt_c = chkp.tile([P, C], f32, tag="knt_c")
    krt_c = chkp.tile([P, C], f32, tag="krt_c")
    cit_c = chkp.tile([P, C], f32, tag="cit_c")
    vp_t = chkp.tile([P, C], f32, tag="vp_t")
    O_c = chkp.tile([P, C], f32, tag="O_c")
    wtf_c = chkp.tile([P, NCT, CH], f32, tag="wtf_c")
    wtb_c = chkp.tile([P, NCT, CH], bf16, tag="wtb_c")
    kntb_c = chkp.tile([P, NCT, CH], bf16, tag="kntb_c")
    vld_c = chkp.tile([P, NCT, CH], f32, tag="vld_c")
    wtlo = chkp.tile([64, NCT, CH], f32, tag="wtlo")
    rtlo = chkp.tile([64, NCT, CH], f32, tag="rtlo")
    cl_al = chkp.tile([64, 2, NCT], f32, tag="cl_al")
    bs_t = chkp.tile([P, 16], f32, tag="bs_t")
    stats6 = chkp.tile([P, 16, 6], f32, tag="stats6")
    mv2 = chkp.tile([P, 16, 2], f32, tag="mv2")
    rstd = chkp.tile([P, 16], f32, tag="rstd")

    tc.strict_bb_all_engine_barrier()

    def col(vn, i):
        return vsb[vn][:, i:i + 1]

    def lerp_into(dst, i, mu_name):
        d = jit3.tile([P, SUP], f32, tag="diff")
        nc.vector.tensor_sub(d, xext[:, i, 0:SUP], xext[:, i, 1:SUP + 1])
        nc.vector.scalar_tensor_tensor(
            out=dst, in0=d, scalar=col(mu_name, i), in1=xext[:, i, 1:SUP + 1],
            op0=OP.mult, op1=OP.add)

    for sp in range(NSUP):
        t0 = sp * SUP
        # ---- x load (t-layout halves) + PE transpose into xext ----
        for i in range(NCT):
            if sp == 0:
                nc.vector.memset(xext[:, i, 0:1], 0.0)
            else:
                nc.vector.tensor_copy(xext[:, i, 0:1], xext[:, i, SUP:SUP + 1])
        for g in range(SUP // P):
            for ih in range(2):
                xt = jit2.tile([P, C // 2], f32, tag="xtld")
                nc.sync.dma_start(
                    out=xt, in_=x_h[t0 + P * g:t0 + P * (g + 1),
                                    (C // 2) * ih:(C // 2) * (ih + 1)])
                for ii in range(NCT // 2):
                    i = (NCT // 2) * ih + ii
                    pt = ps_small.tile([P, P], f32, tag="ptr")
                    nc.tensor.transpose(pt, xt[:, P * ii:P * (ii + 1)], ident)
                    nc.scalar.copy(out=xext[:, i, 1 + P * g:1 + P * (g + 1)], in_=pt)
        # ---- lora A passes ----
        for n, mu in (("iclr_A", "mu_a"), ("decay_A", "mu_d"), ("gate_A", "mu_g")):
            pla = ps_lora.tile([LORA, SUP], f32, tag="pl")
            for i in range(NCT):
                laj = jit2.tile([P, LORA], f32r, tag="laj")
                nc.gpsimd.dma_start(out=laj, in_=la_h[n][P * i:P * (i + 1), :])
                xlo = jit2.tile([P, SUP], f32r, tag="xl")
                lerp_into(xlo, i, mu)
                nc.tensor.matmul(pla, laj, xlo,
                                 start=(i == 0), stop=(i == NCT - 1))
            nc.scalar.copy(out=la_out[n], in_=pla)
        # ---- lora B + activations ----
        for co in range(NCT):
            lb_i = jit2.tile([LORA, P], f32r, tag="lbi")
            nc.gpsimd.dma_start(out=lb_i, in_=lb_h["iclr_B"][:, P * co:P * (co + 1)])
            pib = ps_lora.tile([P, SUP], f32, tag="pl")
            nc.tensor.matmul(pib, lb_i, la_out["iclr_A"], start=True, stop=True)
            nc.scalar.activation(out=aT[:, co, :], in_=pib, func=AF.Sigmoid,
                                 bias=col("iclr_bias", co), scale=1.0)
            lb_d = jit2.tile([LORA, P], f32r, tag="lbi")
            nc.gpsimd.dma_start(out=lb_d, in_=lb_h["decay_B"][:, P * co:P * (co + 1)])
            pdb = ps_lora.tile([P, SUP], f32, tag="pl")
            nc.tensor.matmul(pdb, lb_d, la_out["decay_A"], start=True, stop=True)
            tmp = jit2.tile([P, SUP], f32, tag="acttmp")
            nc.scalar.activation(out=tmp, in_=pdb, func=AF.Tanh,
                                 bias=col("decay_bias", co), scale=1.0)
            nc.scalar.activation(out=tmp, in_=tmp, func=AF.Sigmoid)
            nc.scalar.activation(out=dT[:, co, :], in_=tmp, func=AF.Exp,
                                 scale=-DECAY_SCALE)
            lb_g = jit2.tile([LORA, P], f32r, tag="lbi")
            nc.gpsimd.dma_start(out=lb_g, in_=lb_h["gate_B"][:, P * co:P * (co + 1)])
            pgb = ps_lora.tile([P, SUP], f32, tag="pl")
            nc.tensor.matmul(pgb, lb_g, la_out["gate_A"], start=True, stop=True)
            nc.scalar.activation(out=gateT[:, co, :], in_=pgb, func=AF.Sigmoid)
        # ---- big projections (W_r resident; W_k/W_v streamed) ----
        for pn, mu in (("W_r", "mu_r"), ("W_k", "mu_k"), ("W_v", "mu_v")):
            for cop in range(4):
                pps = [ps_proj.tile([P, SUP], f32, tag="pp", name="pp")
                       for _ in range(2)]
                for i in range(NCT):
                    if pn == "W_r":
                        wtile = wr_sb[:, i, 256 * cop:256 * (cop + 1)]
                    else:
                        wtile = jit2.tile([P, 256], f32r, tag="wstr")
                        nc.gpsimd.dma_start(
                            out=wtile, in_=w_h[pn][P * i:P * (i + 1),
                                                   256 * cop:256 * (cop + 1)])
                    xl = jit2.tile([P, SUP], f32r, tag="xl")
                    lerp_into(xl, i, mu)
                    for cc in range(2):
                        nc.tensor.matmul(
                            pps[cc], wtile[:, P * cc:P * (cc + 1)], xl,
                            start=(i == 0), stop=(i == NCT - 1))
                for cc in range(2):
                    co = 2 * cop + cc
                    pslice = pps[cc]
                    if pn == "W_r":
                        nc.scalar.copy(out=rT[:, co, :], in_=pslice)
                    elif pn == "W_v":
                        nc.scalar.copy(out=vT[:, co, :], in_=pslice)
                    else:
                        nc.vector.tensor_scalar_mul(
                            out=knT[:, co, :], in0=pslice,
                            scalar1=col("removal_key_multiplier", co))
                        f = jit2.tile([P, SUP], f32, tag="fmix")
                        nc.vector.tensor_scalar(
                            out=f, in0=aT[:, co, :], scalar1=col("iclr_mix_amt", co),
                            scalar2=ommix[:, co:co + 1], op0=OP.mult, op1=OP.add)
                        nc.vector.tensor_mul(krT[:, co, :], pslice, f)
        # ---- removal-key norm + bonus pack ----
        pnb = ps_small.tile([16, 2 * SUP], f32, tag="ptr")
        for i in range(NCT):
            nsq = jit1.tile([P, 2 * SUP], f32r, tag="nsq")
            nc.vector.tensor_mul(nsq[:, 0:SUP], knT[:, i, :], knT[:, i, :])
            z2f = jit1.tile([P, SUP], f32, tag="z2f")
            nc.gpsimd.tensor_mul(z2f, rT[:, i, :], krT[:, i, :])
            nc.vector.tensor_scalar_mul(out=nsq[:, SUP:2 * SUP], in0=z2f,
                                        scalar1=col("bonus_multiplier", i))
            nc.tensor.matmul(pnb, inds[i], nsq, start=(i == 0), stop=(i == NCT - 1))
        nc.scalar.copy(out=nb, in_=pnb)
        nc.scalar.activation(out=nb[:, 0:SUP], in_=nb[:, 0:SUP], func=AF.Sqrt)
        nc.vector.tensor_scalar_max(out=nb[:, 0:SUP], in0=nb[:, 0:SUP],
                                    scalar1=NORM_EPS)
        nc.vector.reciprocal(out=nb[:, 0:SUP], in_=nb[:, 0:SUP])
        for i in range(NCT):
            rnb = jit1.tile([P, SUP], f32, tag="rnb")
            src = nb[2 * i:2 * i + 2, 0:SUP]
            nc.sync.dma_start(out=rnb, in_=bass.AP(
                tensor=src.tensor, offset=src.offset,
                ap=[src.ap[0], [0, 64], src.ap[1]]))
            nc.vector.tensor_mul(knT[:, i, :], knT[:, i, :], rnb)

        # ================= WKV chunks =================
        for ch in range(NCH):
            cs = ch * CH
            row = t0 + cs
            for i in range(NCT):
                nc.vector.memset(cext[:, i, 0:1], 1.0)
                nc.vector.tensor_tensor_scan(
                    out=cext[:, i, 1:CH + 1], data0=dT[:, i, cs:cs + CH],
                    data1=zeros, initial=1.0, op0=OP.mult, op1=OP.max)
                ci = jit2.tile([P, CH], f32, tag="ci")
                nc.vector.reciprocal(out=ci, in_=cext[:, i, 1:CH + 1])
                for srcT, dstt in ((knT[:, i, cs:cs + CH], knt_c[:, P * i:P * (i + 1)]),
                                   (krT[:, i, cs:cs + CH], krt_c[:, P * i:P * (i + 1)]),
                                   (vT[:, i, cs:cs + CH], vp_t[:, P * i:P * (i + 1)]),
                                   (ci, cit_c[:, P * i:P * (i + 1)])):
                    pt = ps_small.tile([P, P], f32, tag="ptr")
                    nc.tensor.transpose(pt, srcT, ident)
                    nc.scalar.copy(out=dstt, in_=pt)
                nc.vector.tensor_mul(wtf_c[:, i, :], knT[:, i, cs:cs + CH],
                                     cext[:, i, 0:CH])
                nc.vector.tensor_mul(wtf_c[:, i, :], wtf_c[:, i, :],
                                     aT[:, i, cs:cs + CH])
                nc.vector.tensor_copy(out=wtb_c[:, i, :], in_=wtf_c[:, i, :])
                nc.vector.tensor_mul(kntb_c[:, i, :], knT[:, i, cs:cs + CH], ci)
                nc.vector.tensor_mul(vld_c[:, i, :], vT[:, i, cs:cs + CH], ci)
            for i in range(NCT):
                nc.sync.dma_start(out=wtlo[:, i, :], in_=wtf_c[64:128, i, :])
                nc.sync.dma_start(out=rtlo[:, i, :], in_=rT[64:128, i, cs:cs + CH])
            nc.sync.dma_start(out=cl_al[:, 0, :], in_=cext[0:64, :, CH:CH + 1])
            nc.sync.dma_start(out=cl_al[:, 1, :], in_=cext[64:128, :, CH:CH + 1])
            nc.sync.dma_start(out=vp_h[row:row + CH, :], in_=vp_t)
            for h in range(H):
                i, hh = h // 2, h % 2
                ns = slice(64 * hh, 64 * (hh + 1))
                cn = slice(P * i + 64 * hh, P * i + 64 * (hh + 1))
                RT = rT[:, i, cs:cs + CH][ns, :]
                RT0 = rtlo[:, i, :] if hh else rT[0:64, i, cs:cs + CH]
                WT0 = wtlo[:, i, :] if hh else wtf_c[0:64, i, :]
                Svw, Stvw = S_st[:, i, hh, :], St_st[:, i, hh, :]
                cl_col = cl_al[:, hh, i:i + 1]
                pg = ps_wkv.tile([P, P], f32, tag="pwk")
                nc.tensor.matmul(pg, kntb_c[ns, i, :], wtb_c[ns, i, :],
                                 start=True, stop=True)
                Gu = pairp.tile([P, P], bf16, tag="Gu")
                nc.vector.tensor_mul(Gu, pg, mask_su)
                pa = ps_wkv.tile([P, P], f32, tag="pwk")
                nc.tensor.matmul(pa, vld_c[ns, i, :], wtf_c[ns, i, :],
                                 start=True, stop=True)
                Av = pairp.tile([P, P], f32, tag="Av")
                nc.vector.tensor_mul(Av, pa, mask_su)
                pp2 = ps_wkv.tile([P, P], f32, tag="pwk")
                nc.tensor.matmul(pp2, krT[:, i, cs:cs + CH][ns, :], RT,
                                 start=True, stop=True)
                Pm = pairp.tile([P, P], f32, tag="Pm")
                nc.vector.tensor_mul(Pm, pp2, mask_ui)
                pb = ps_wkv.tile([P, 64], f32, tag="pwk")
                nc.tensor.matmul(pb, WT0, Svw, start=True, stop=False)
                nc.tensor.matmul(pb, Av, krt_c[:, cn], start=False, stop=True)
                Bt = pairp.tile([P, 64], f32, tag="Bt")
                nc.scalar.copy(out=Bt, in_=pb)
                Xb = pairp.tile([P, 64], bf16, tag="Xb")
                nc.vector.tensor_copy(out=Xb, in_=pb)
                Um = None
                for it in range(NEUMANN_K):
                    px = ps_wkv.tile([P, 64], f32, tag="pwk")
                    nc.tensor.matmul(px, Gu, Xb, start=True, stop=True)
                    if it < NEUMANN_K - 1:
                        Xb = pairp.tile([P, 64], bf16, tag="Xb")
                        nc.vector.tensor_sub(Xb, Bt, px)
                    else:
                        Um = pairp.tile([P, 64], f32, tag="Um")
                        nc.vector.tensor_sub(Um, px, Bt)
                put = ps_small.tile([64, P], f32, tag="ptr", name="put")
                nc.tensor.transpose(put, Um, ident)
                Utf = pairp.tile([64, P], f32, tag="Ut")
                nc.scalar.copy(out=Utf, in_=put)
                pq = ps_wkv.tile([P, P], f32, tag="pwk")
                nc.tensor.matmul(pq, Utf, RT0, start=True, stop=True)
                Qm = pairp.tile([P, P], f32, tag="Qm")
                nc.vector.tensor_mul(Qm, pq, mask_ui)
                Vld = pairp.tile([P, 64], f32, tag="Vld")
                nc.vector.tensor_mul(Vld, vp_t[:, cn], cit_c[:, cn])
                Knl = pairp.tile([P, 64], f32, tag="Knl")
                nc.vector.tensor_mul(Knl, knt_c[:, cn], cit_c[:, cn])
                ctp = pairp.tile([P, 64], f32, tag="ctp")
                nc.vector.reciprocal(out=ctp, in_=cit_c[:, cn])
                po = ps_wkv.tile([P, 64], f32, tag="pwk")
                nc.tensor.matmul(po, RT0, Stvw, start=True, stop=False)
                nc.tensor.matmul(po, Pm, Vld, start=False, stop=False)
                nc.tensor.matmul(po, Qm, Knl, start=False, stop=True)
                nc.vector.tensor_mul(O_c[:, cn], po, ctp)
                pst = ps_small.tile([64, 64], f32, tag="ptr", name="pst")
                nc.tensor.matmul(pst, Vld, krt_c[:, cn], start=True, stop=False)
                nc.tensor.matmul(pst, Knl, Um, start=False, stop=True)
                nc.vector.tensor_add(Svw, Svw, pst)
                nc.vector.tensor_scalar_mul(out=Svw, in0=Svw, scalar1=cl_col)
                pstt = ps_small.tile([64, 64], f32, tag="ptr", name="pstt")
                nc.tensor.transpose(pstt, Svw, ident[0:64, 0:64])
                nc.scalar.copy(out=Stvw, in_=pstt)
            # ---- bonus + GroupNorm (t-layout) ----
            pbs = ps_small.tile([P, 16], f32, tag="ptr")
            nc.tensor.transpose(pbs, nb[:, SUP + cs:SUP + cs + CH],
                                ident[0:16, 0:16])
            nc.scalar.copy(out=bs_t, in_=pbs)
            for h in range(H):
                cn = slice(64 * h, 64 * (h + 1))
                nc.vector.scalar_tensor_tensor(
                    out=O_c[:, cn], in0=vp_t[:, cn], scalar=bs_t[:, h:h + 1],
                    in1=O_c[:, cn], op0=OP.mult, op1=OP.add)
                nc.vector.bn_stats(out=stats6[:, h, :], in_=O_c[:, cn])
                nc.vector.bn_aggr(out=mv2[:, h, :], in_=stats6[:, h, :])
            nc.scalar.activation(out=rstd, in_=mv2[:, :, 1], func=AF.Sqrt,
                                 bias=gn_eps)
            nc.vector.reciprocal(out=rstd, in_=rstd)
            for h in range(H):
                cn = slice(64 * h, 64 * (h + 1))
                nc.vector.tensor_scalar(
                    out=O_c[:, cn], in0=O_c[:, cn], scalar1=mv2[:, h, 0:1],
                    scalar2=rstd[:, h:h + 1], op0=OP.subtract, op1=OP.mult)
            # ---- ln/gate -> yT super staging (f32r) ----
            for i in range(NCT):
                pzt = ps_small.tile([P, P], f32, tag="ptr")
                nc.tensor.transpose(pzt, O_c[:, P * i:P * (i + 1)], ident)
                yt1 = jit2.tile([P, P], f32, tag="yt1")
                nc.vector.tensor_scalar(
                    out=yt1, in0=pzt, scalar1=col("ln_w", i),
                    scalar2=col("ln_b", i), op0=OP.mult, op1=OP.add)
                nc.vector.tensor_mul(yTs[:, i, cs:cs + CH], yt1,
                                     gateT[:, i, cs:cs + CH])
        # ---- W_o (per super, streamed weights) ----
        for half in range(2):
            pys = [ps_proj.tile([P, C // 2], f32, tag="pp", name="pyo")
                   for _ in range(NCH)]
            for i in range(NCT):
                wo = jit2.tile([P, C // 2], f32r, tag="wstr")
                nc.gpsimd.dma_start(
                    out=wo, in_=w_h["W_o"][P * i:P * (i + 1),
                                           (C // 2) * half:(C // 2) * (half + 1)])
                for chh in range(NCH):
                    nc.tensor.matmul(pys[chh], yTs[:, i, CH * chh:CH * (chh + 1)],
                                     wo, start=(i == 0), stop=(i == NCT - 1))
            for chh in range(NCH):
                yst = jit2.tile([P, C // 2], f32, tag="yst")
                nc.scalar.copy(out=yst, in_=pys[chh])
                nc.sync.dma_start(
                    out=y_h[t0 + CH * chh:t0 + CH * (chh + 1),
                            (C // 2) * half:(C // 2) * (half + 1)],
                    in_=yst)
    ctx.close()


def kernel(**inputs):
    from concourse.bass_utils import run_bass_kernel_spmd
    if "nc" not in _CACHE:
        _CACHE["nc"] = _build()
    nc = _CACHE["nc"]
    x = np.ascontiguousarray(np.asarray(inputs["x"], dtype=np.float32))
    shared = {n: np.ascontiguousarray(np.asarray(inputs[n], dtype=np.float32))
              for n in MAT_NAMES + VEC_NAMES}
    in_maps = [dict(shared, x=np.ascontiguousarray(x[b])) for b in range(B)]
    res = run_bass_kernel_spmd(nc, in_maps, core_ids=list(range(B)))
    y = np.stack([res.results[b]["y"] for b in range(B)])
    vp = np.stack([res.results[b]["vp"] for b in range(B)])
    return y, vp

